# revision 1
# baseline (speedup 1.0000x reference)
"""Self-contained Trainium2 Bass kernel for nn_Attn_3375844295368.

Reference computation (per batch b):
    normed = LayerNorm(x[b])                      # (4096, 512)
    qk = silu(normed @ W.T + bias)                # (4096, 128)
    q = (qk*g0 + b0) / sqrt(N); k = qk*g1 + b1
    sim = q @ k.T                                 # (4096, 4096)
    attn = relu(sim)^2 / (rowsum + 1e-6)

Sharding: 8 cores = 4 batches x 2 query-halves.  Each core receives the
full x[b] transposed (dim-major, bf16) and rolled so its query half is
always columns 0..2047 -> all cores run one identical SPMD graph,
outputs are un-rolled on the host.

On-chip layout trick: LayerNorm stats are computed with an all-ones
(1/512) stationary matmul, which both reduces over the feature dim and
broadcasts the result to all 128 partitions.  The LN mean is folded into
the qk linear as a rank-1 PSUM accumulation (svec (x) -mu) and the rstd
column scale is applied after the linear (on 128 rows instead of 512).
"""

import sys

sys.path.insert(0, "/opt/trn_rl_repo")

import numpy as np
import ml_dtypes

import concourse.bass as bass
import concourse.bacc as bacc
import concourse.tile as tile
from concourse import mybir
from concourse.bass_utils import run_bass_kernel_spmd

B, N, DIM, QK = 4, 4096, 512, 128
NCORES = 8
HALF = N // 2
P = 128
NT = N // 512  # 8 column tiles of 512
NQB = HALF // P  # 16 query blocks per core
LN_EPS = 1e-5
DEN_EPS = 1e-6
F32 = mybir.dt.float32
BF16 = mybir.dt.bfloat16
BF16_NP = ml_dtypes.bfloat16

_CACHE = {}


def build_bass(reps=1, dyn_reps=False, r2_bufs=3, out_bufs=4, rsb_bufs=4,
               norm_act_frac=0, sq_on_act=False, z_bufs=4, st_bufs=2):
    nc = bacc.Bacc()
    xT = nc.declare_dram_parameter("xT", [DIM, N], BF16, isOutput=False)
    wT = nc.declare_dram_parameter("wT", [DIM, QK], BF16, isOutput=False)
    svec = nc.declare_dram_parameter("svec", [1, P], BF16, isOutput=False)
    biasf = nc.declare_dram_parameter("biasf", [P, 1], F32, isOutput=False)
    aff = nc.declare_dram_parameter("aff", [P, 4], F32, isOutput=False)
    if dyn_reps:
        nreps = nc.declare_dram_parameter("nreps", [1, 1], mybir.dt.int32,
                                          isOutput=False)
    out = nc.declare_dram_parameter("out", [HALF, N], F32, isOutput=True)

    AF = mybir.ActivationFunctionType
    OP = mybir.AluOpType

    with tile.TileContext(nc) as tc:
        with tc.tile_pool(name="consts", bufs=1) as consts, \
             tc.tile_pool(name="xin", bufs=1) as xinp, \
             tc.tile_pool(name="xsq", bufs=6) as xsqp, \
             tc.tile_pool(name="st_sb", bufs=2) as stsb, \
             tc.tile_pool(name="rsb", bufs=rsb_bufs) as rsbp, \
             tc.tile_pool(name="r2p", bufs=r2_bufs) as r2p, \
             tc.tile_pool(name="accp", bufs=4) as accp, \
             tc.tile_pool(name="outp", bufs=out_bufs) as outp:
            wts = consts.tile([P, 4, QK], BF16)
            nc.sync.dma_start(out=wts, in_=wT.rearrange("(c p) m -> p c m", p=P))
            onest = consts.tile([P, P], BF16)
            nc.vector.memset(onest, 1.0 / DIM)
            svect = consts.tile([1, P], BF16)
            nc.sync.dma_start(out=svect, in_=svec[:])
            biast = consts.tile([P, 1], F32)
            nc.sync.dma_start(out=biast, in_=biasf[:])
            afft = consts.tile([P, 4], F32)
            nc.sync.dma_start(out=afft, in_=aff[:])
            epst = consts.tile([P, 1], F32)
            nc.vector.memset(epst, LN_EPS)
            kT = consts.tile([P, N], BF16)
            qT = consts.tile([P, HALF], BF16)

            xins = []
            for c in range(4):
                xi = xinp.tile([P, N], BF16, tag=f"xin{c}")
                nc.sync.dma_start(out=xi, in_=xT[c * P:(c + 1) * P, :])
                xins.append(xi)

            _rep_cm = None
            if dyn_reps:
                nrt = consts.tile([1, 1], mybir.dt.int32)
                nc.sync.dma_start(out=nrt, in_=nreps[:])
                _regs = bass.RegisterHandles([
                    nc.engines[e].alloc_register(f"nreps_{e.name}")
                    for e in mybir.ALL_ENGINES])
                nc.regs_load(_regs, nrt[0:1, 0:1])
                rv = nc.snap(_regs, min_val=1, max_val=1024)
                _rep_cm = tc.For_i(0, rv, 1,
                                   hint_engines=(mybir.EngineType.PE,
                                                 mybir.EngineType.DVE,
                                                 mybir.EngineType.Activation))
                _rep_cm.__enter__()
            for _rep in range(reps):
              # ---------------- phase 1: LN + linear + silu + offsetscale --
              with tc.tile_pool(name="st_ps", bufs=st_bufs, space="PSUM") as stps, \
                   tc.tile_pool(name="z_ps", bufs=z_bufs, space="PSUM") as zps:
                  for t in range(NT):
                      sl = slice(t * 512, (t + 1) * 512)
                      # mean & E[x^2]; the all-ones(1/512) stationary both
                      # reduces over dim and broadcasts to all partitions
                      mu_ps = stps.tile([P, 512], F32)
                      s2_ps = stps.tile([P, 512], F32)
                      for c in range(4):
                          nc.tensor.matmul(mu_ps, onest, xins[c][:, sl],
                                           start=(c == 0), stop=(c == 3))
                      for c in range(4):
                          xq = xsqp.tile([P, 512], BF16)
                          if sq_on_act:
                              nc.scalar.activation(xq, xins[c][:, sl],
                                                   AF.Square)
                          else:
                              nc.vector.tensor_mul(xq, xins[c][:, sl],
                                                   xins[c][:, sl])
                          nc.tensor.matmul(s2_ps, onest, xq,
                                           start=(c == 0), stop=(c == 3))
                      # z = W' @ x  (+ svec (x) -mu rank-1 LN-mean fold)
                      z_ps = zps.tile([P, 512], F32)
                      for c in range(4):
                          nc.tensor.matmul(z_ps, wts[:, c, :], xins[c][:, sl],
                                           start=(c == 0), stop=False)
                      negmu = stsb.tile([P, 512], BF16)
                      nc.vector.tensor_scalar_mul(negmu, mu_ps, -1.0)
                      musq = stsb.tile([P, 512], BF16)
                      nc.vector.tensor_mul(musq, negmu, negmu)
                      var = stsb.tile([P, 512], F32)
                      nc.vector.scalar_tensor_tensor(
                          out=var, in0=s2_ps, scalar=1.0, in1=musq,
                          op0=OP.mult, op1=OP.subtract)
                      stdv = stsb.tile([P, 512], F32)
                      nc.scalar.activation(stdv, var, AF.Sqrt,
                                           bias=epst, scale=1.0)
                      rstd = stsb.tile([P, 512], F32)
                      nc.vector.reciprocal_approx_fast(out=rstd, in_=stdv)
                      nc.tensor.matmul(z_ps, svect, negmu[0:1, :],
                                       start=False, stop=True)
                      qksc = stsb.tile([P, 512], F32)
                      nc.vector.scalar_tensor_tensor(
                          out=qksc, in0=z_ps, scalar=1.0, in1=rstd,
                          op0=OP.mult, op1=OP.mult)
                      qka = stsb.tile([P, 512], F32)
                      nc.scalar.activation(qka, qksc, AF.Silu,
                                           bias=biast, scale=1.0)
                      nc.vector.tensor_scalar(
                          out=kT[:, sl], in0=qka,
                          scalar1=afft[:, 2:3], scalar2=afft[:, 3:4],
                          op0=OP.mult, op1=OP.add)
                      if t < NT // 2:
                          nc.vector.tensor_scalar(
                              out=qT[:, sl], in0=qka,
                              scalar1=afft[:, 0:1], scalar2=afft[:, 1:2],
                              op0=OP.mult, op1=OP.add)

              # ---------------- phase 2: attention + relu^2 row-normalize ----
              with tc.tile_pool(name="sim_ps", bufs=2, space="PSUM") as simps:
                  for qb in range(NQB):
                      r2 = r2p.tile([P, N], BF16)
                      accs = accp.tile([P, 2], F32)
                      for h in range(2):
                          hsl = slice(h * 2048, (h + 1) * 2048)
                          sim = simps.tile([P, 2048], F32)
                          for n in range(4):
                              nc.tensor.matmul(
                                  sim[:, n * 512:(n + 1) * 512],
                                  qT[:, qb * P:(qb + 1) * P],
                                  kT[:, h * 2048 + n * 512:
                                        h * 2048 + (n + 1) * 512],
                                  start=True, stop=True)
                          rr = rsbp.tile([P, 2048], BF16)
                          nc.scalar.activation(rr, sim, AF.Relu)
                          nc.vector.scalar_tensor_tensor(
                              out=r2[:, hsl], in0=rr, scalar=0.0, in1=rr,
                              op0=OP.add, op1=OP.mult,
                              accum_out=accs[:, h:h + 1])
                      rden = accp.tile([P, 1], F32)
                      nc.vector.tensor_reduce(
                          out=rden, in_=accs, axis=mybir.AxisListType.X,
                          op=OP.add)
                      nc.vector.tensor_scalar_add(out=rden, in0=rden,
                                                  scalar1=DEN_EPS)
                      rcp = accp.tile([P, 1], F32)
                      nc.vector.reciprocal_approx_fast(out=rcp, in_=rden)
                      ot = outp.tile([P, N], F32)
                      if (qb % 4) < norm_act_frac:
                          nc.scalar.activation(ot, r2, AF.Copy,
                                               bias=0.0, scale=rcp)
                      else:
                          nc.vector.tensor_scalar_mul(out=ot, in0=r2,
                                                      scalar1=rcp)
                      nc.sync.dma_start(out=out[qb * P:(qb + 1) * P, :],
                                        in_=ot)
            if _rep_cm is not None:
                _rep_cm.__exit__(None, None, None)
    nc.compile()
    return nc


def _prepare_in_maps(x, ln_w, ln_b, w_qk, b_qk, gamma, beta):
    x = np.asarray(x, np.float32)
    ln_w = np.asarray(ln_w, np.float32)
    ln_b = np.asarray(ln_b, np.float32)
    w_qk = np.asarray(w_qk, np.float32)
    b_qk = np.asarray(b_qk, np.float32)
    gamma = np.asarray(gamma, np.float32)
    beta = np.asarray(beta, np.float32)

    wp = (w_qk * ln_w[None, :]).astype(np.float64)
    bias_fold = (b_qk.astype(np.float64) + wp @ ln_b.astype(np.float64))
    svec = wp.sum(axis=1)  # (128,)
    scale = 1.0 / np.sqrt(np.float64(N))
    aff = np.stack([gamma[0] * scale, beta[0] * scale, gamma[1], beta[1]],
                   axis=1).astype(np.float32)  # (128, 4)

    wT = np.ascontiguousarray(wp.T).astype(BF16_NP)  # (512, 128)
    svec_bf = svec.astype(BF16_NP).reshape(1, P)
    bias_f = bias_fold.astype(np.float32).reshape(P, 1)

    in_maps = []
    for c in range(NCORES):
        b, h = c // 2, c % 2
        xt = x[b].T
        if h:
            xt = np.roll(xt, -HALF, axis=1)
        xt = np.ascontiguousarray(xt).astype(BF16_NP)
        in_maps.append({
            "xT": xt,
            "wT": wT,
            "svec": svec_bf,
            "biasf": bias_f,
            "aff": aff,
        })
    return in_maps


def _run(in_maps, trace=False):
    if "nc" not in _CACHE:
        _CACHE["nc"] = build_bass()
    nc = _CACHE["nc"]
    res = run_bass_kernel_spmd(nc, in_maps, core_ids=list(range(NCORES)),
                               trace=trace)
    return res


def kernel(x, ln_w, ln_b, w_qk, b_qk, gamma, beta, _trace=False):
    in_maps = _prepare_in_maps(x, ln_w, ln_b, w_qk, b_qk, gamma, beta)
    res = _run(in_maps, trace=_trace)
    out = np.empty((B, N, N), np.float32)
    for c in range(NCORES):
        b, h = c // 2, c % 2
        o = np.asarray(res.results[c]["out"], np.float32)
        if h:
            o = np.roll(o, HALF, axis=1)
        out[b, h * HALF:(h + 1) * HALF, :] = o
    if _trace:
        return out, res
    return out



# revision 31
# speedup vs baseline: 72.0712x; 72.0712x over previous
"""Self-contained Trainium2 Bass kernel for nn_Attn_3375844295368.

Reference computation (per batch b):
    normed = LayerNorm(x[b])                      # (4096, 512)
    qk = silu(normed @ W.T + bias)                # (4096, 128)
    q = (qk*g0 + b0) / sqrt(N); k = qk*g1 + b1
    sim = q @ k.T                                 # (4096, 4096)
    attn = relu(sim)^2 / (rowsum + 1e-6)

Sharding: 8 cores = 4 batches x 2 query-halves.  Each core receives the
full x[b] transposed (dim-major, bf16) and rolled so its query half is
always columns 0..2047 -> all cores run one identical SPMD graph,
outputs are un-rolled on the host.

Design (v2, engine-balanced, bf16 output):
  - Output is stored as bf16 (halves the dominant HBM store traffic);
    the host upcasts to f32.  DMA floor/core: 16.8MB out + 4.2MB in
    at ~360GB/s  ~= 63us/rep.
  - Phase 2 PSUM drain split across engines: DVE does relu^2+rowsum in
    a single scalar_tensor_tensor (max0, mult, accum) on quarter 0 of
    each 128-row query block; Act relu's quarters 1..3 (PSUM f32 ->
    SBUF bf16); Pool (no PSUM port!) squares+accumulates those in
    place.  Row normalization: rowsum+eps reduce on Pool, reciprocal
    on DVE, scale as a 4x-mode DVE tensor_scalar (bf16 all-SBUF).
  - Phase 1 (LN stats via all-ones matmul broadcast trick, mean folded
    into the linear as a rank-1 update, rstd applied post-linear) is
    spread: DVE negmu/var/qksc + kT/qT offsetscale (4x), Act rstd via
    Abs_reciprocal_sqrt + silu, Pool x^2 tiles and mu^2.
  - PSUM: stats 2 banks + z 2 banks + sim quarters [P,1024]x2 bufs
    4 banks = 8 banks total, all pools open for the whole kernel so
    consecutive reps (and phases) overlap.
"""

import sys

sys.path.insert(0, "/opt/trn_rl_repo")

import numpy as np
import ml_dtypes

import concourse.bass as bass
import concourse.bacc as bacc
import concourse.tile as tile
from concourse import mybir
from concourse.bass_utils import run_bass_kernel_spmd

B, N, DIM, QK = 4, 4096, 512, 128
NCORES = 8
HALF = N // 2
P = 128
NT = N // 512  # 8 column tiles of 512
NQB = HALF // P  # 16 query blocks per core
NQ = 4  # sim quarters per block
QW = N // NQ  # 1024 cols per quarter
LN_EPS = 1e-5
DEN_EPS = 1e-6
F32 = mybir.dt.float32
BF16 = mybir.dt.bfloat16
BF16_NP = ml_dtypes.bfloat16

_CACHE = {}


def build_bass(reps=1, dyn_reps=False, use_ars=True,
               xq_eng=("dve", "dve", "pool", "pool"),
               blk_pats=(("aa", "ad"), ("da", "ad")),
               r2_bufs=3, out_bufs=3, xload_chunks=4):
    nc = bacc.Bacc()
    xT = nc.declare_dram_parameter("xT", [DIM, N], BF16, isOutput=False)
    wT = nc.declare_dram_parameter("wT", [DIM, QK], BF16, isOutput=False)
    svec = nc.declare_dram_parameter("svec", [1, P], BF16, isOutput=False)
    biasf = nc.declare_dram_parameter("biasf", [P, 1], F32, isOutput=False)
    aff = nc.declare_dram_parameter("aff", [P, 4], F32, isOutput=False)
    if dyn_reps:
        nreps = nc.declare_dram_parameter("nreps", [1, 1], mybir.dt.int32,
                                          isOutput=False)
    out = nc.declare_dram_parameter("out", [HALF, N], BF16, isOutput=True)

    AF = mybir.ActivationFunctionType
    OP = mybir.AluOpType

    with tile.TileContext(nc) as tc:
        with tc.tile_pool(name="consts", bufs=1) as consts, \
             tc.tile_pool(name="xin", bufs=1) as xinp, \
             tc.tile_pool(name="xsq", bufs=6) as xsqp, \
             tc.tile_pool(name="qksc", bufs=1) as qkscp, \
             tc.tile_pool(name="st_sb", bufs=4) as stsb, \
             tc.tile_pool(name="accp", bufs=4) as accp, \
             tc.tile_pool(name="r2p", bufs=r2_bufs) as r2p, \
             tc.tile_pool(name="outp", bufs=out_bufs) as outp:
            wts = consts.tile([P, 4, QK], BF16)
            nc.sync.dma_start(out=wts, in_=wT.rearrange("(c p) m -> p c m", p=P))
            onest = consts.tile([P, P], BF16)
            nc.vector.memset(onest, 1.0 / DIM)
            svect = consts.tile([1, P], BF16)
            nc.sync.dma_start(out=svect, in_=svec[:])
            nones1 = consts.tile([1, P], BF16)
            nc.vector.memset(nones1, -1.0)
            biast = consts.tile([P, 1], F32)
            nc.sync.dma_start(out=biast, in_=biasf[:])
            afft = consts.tile([P, 4], F32)
            nc.sync.dma_start(out=afft, in_=aff[:])
            epst = consts.tile([P, 1], F32)
            nc.vector.memset(epst, LN_EPS)
            dent = consts.tile([P, 1], F32)
            nc.vector.memset(dent, DEN_EPS)
            zerot = consts.tile([P, 1], F32)
            nc.vector.memset(zerot, 0.0)
            kT = consts.tile([P, N], BF16)
            qT = consts.tile([P, HALF], BF16)

            _rep_cm = None
            if dyn_reps:
                nrt = consts.tile([1, 1], mybir.dt.int32)
                nc.sync.dma_start(out=nrt, in_=nreps[:])
                _regs = bass.RegisterHandles([
                    nc.engines[e].alloc_register(f"nreps_{e.name}")
                    for e in mybir.ALL_ENGINES])
                nc.regs_load(_regs, nrt[0:1, 0:1])
                rv = nc.snap(_regs, min_val=1, max_val=1024)
                _rep_cm = tc.For_i(0, rv, 1,
                                   hint_engines=(mybir.EngineType.PE,
                                                 mybir.EngineType.DVE,
                                                 mybir.EngineType.Activation,
                                                 mybir.EngineType.Pool))
                _rep_cm.__enter__()
            for _rep in range(reps):
                # x input: 4 dim-chunks x 2 column halves, interleaved so the
                # first phase-1 tiles can start after ~1/2 of the load.
                xins = []
                for c in range(4):
                    xi = xinp.tile([P, N], BF16, tag=f"xin{c}")
                    xins.append(xi)
                cw = N // xload_chunks
                for h in range(xload_chunks):
                    for c in range(4):
                        nc.sync.dma_start(
                            out=xins[c][:, h * cw:(h + 1) * cw],
                            in_=xT[c * P:(c + 1) * P, h * cw:(h + 1) * cw])

                # ---------- phase 1A: LN stats + linear (x rstd) ------------
                # Two stages so the Act engine switches function tables only
                # twice per rep (ARS set in A, Silu set in B; Relu for phase
                # 2 is in every set).  Separate 2-buf PSUM pools (6 banks)
                # close before phase 2 opens its 8-bank sim pool; with the
                # For_i all-engine barrier per rep there is no cross-rep
                # overlap to preserve.
                qkscs = []
                with tc.tile_pool(name="mu_ps", bufs=2, space="PSUM") as mups, \
                     tc.tile_pool(name="s2_ps", bufs=2, space="PSUM") as s2ps, \
                     tc.tile_pool(name="z_psp", bufs=2, space="PSUM") as zps:
                    for t in range(NT):
                        sl = slice(t * 512, (t + 1) * 512)
                        # mean & E[x^2]; the all-ones(1/512) stationary both
                        # reduces over dim and broadcasts to all partitions
                        mu_ps = mups.tile([P, 512], F32)
                        s2_ps = s2ps.tile([P, 512], F32)
                        z_ps = zps.tile([P, 512], F32)
                        for c in range(4):
                            nc.tensor.matmul(mu_ps, onest, xins[c][:, sl],
                                             start=(c == 0), stop=(c == 3))
                        for c in range(4):
                            xq = xsqp.tile([P, 512], BF16)
                            if xq_eng[c] == "pool":
                                nc.gpsimd.tensor_mul(xq, xins[c][:, sl],
                                                     xins[c][:, sl])
                            else:
                                nc.vector.tensor_mul(xq, xins[c][:, sl],
                                                     xins[c][:, sl])
                            nc.tensor.matmul(s2_ps, onest, xq,
                                             start=(c == 0), stop=False)
                        # z = W' @ x  (+ svec (x) -mu rank-1 LN-mean fold)
                        for c in range(4):
                            nc.tensor.matmul(z_ps, wts[:, c, :],
                                             xins[c][:, sl],
                                             start=(c == 0), stop=False)
                        negmu = stsb.tile([P, 512], BF16)
                        nc.vector.tensor_scalar_mul(negmu, mu_ps, -1.0)
                        # -mu^2 folded into s2 PSUM as a rank-1 update
                        # (negative all-ones stationary x mu^2 row), so
                        # rstd = ARS(s2_ps + eps) straight from PSUM
                        musqp = stsb.tile([P, 512], BF16)
                        nc.gpsimd.tensor_mul(musqp, negmu, negmu)
                        nc.tensor.matmul(s2_ps, nones1, musqp[0:1, :],
                                         start=False, stop=True)
                        nc.tensor.matmul(z_ps, svect, negmu[0:1, :],
                                         start=False, stop=True)
                        rstd = stsb.tile([P, 512], F32)
                        if use_ars:
                            nc.scalar.activation(rstd, s2_ps,
                                                 AF.Abs_reciprocal_sqrt,
                                                 bias=epst)
                        else:
                            stdv = stsb.tile([P, 512], F32)
                            nc.scalar.activation(stdv, s2_ps, AF.Sqrt,
                                                 bias=epst)
                            nc.vector.reciprocal_approx_fast(out=rstd,
                                                             in_=stdv)
                        qksc = qkscp.tile([P, 512], F32, tag=f"qksc{t}")
                        nc.vector.scalar_tensor_tensor(
                            out=qksc, in0=z_ps, scalar=1.0, in1=rstd,
                            op0=OP.mult, op1=OP.mult)
                        qkscs.append(qksc)
                # ---------- phase 1B: silu + offsetscale --------------------
                # gate (==1.0) depends on the last stage-A qksc so the
                # scheduler cannot interleave Silu with ARS on the Act
                # engine (each interleave costs a 1283ns act-table reload)
                gate = stsb.tile([P, 1], F32)
                nc.vector.tensor_scalar(
                    out=gate, in0=qkscs[NT - 1][:, 0:1],
                    scalar1=0.0, scalar2=1.0, op0=OP.mult, op1=OP.add)
                for t in range(NT):
                    sl = slice(t * 512, (t + 1) * 512)
                    qka = stsb.tile([P, 512], F32)
                    nc.scalar.activation(qka, qkscs[t], AF.Silu,
                                         bias=biast, scale=gate)
                    nc.vector.tensor_scalar(
                        out=kT[:, sl], in0=qka,
                        scalar1=afft[:, 2:3], scalar2=afft[:, 3:4],
                        op0=OP.mult, op1=OP.add)
                    if t < NT // 2:
                        nc.vector.tensor_scalar(
                            out=qT[:, sl], in0=qka,
                            scalar1=afft[:, 0:1], scalar2=afft[:, 1:2],
                            op0=OP.mult, op1=OP.add)

                # ---------- phase 2: attention + relu^2 row-normalize -------
                # Normalize+scale+store of block qb-1 is emitted after block
                # qb's drains (software pipelining): DVE/Pool run in order,
                # so this keeps them from stalling on the cross-engine
                # rowsum -> reciprocal -> scale chain of the current block.
                pending = None

                def _finish(pend):
                    qsl_p, r2_p, accs_p = pend
                    # rowsum + eps via tiny Pool tensor_tensor adds (stt /
                    # tensor_scalar are not implemented on the Q7 Pool
                    # engine), reciprocal + 4x-mode scale on DVE
                    nc.gpsimd.tensor_add(accs_p[:, 4:5], accs_p[:, 0:1],
                                         accs_p[:, 1:2])
                    nc.gpsimd.tensor_add(accs_p[:, 5:6], accs_p[:, 4:5],
                                         dent[:])
                    rcp = accp.tile([P, 1], F32)
                    nc.vector.reciprocal_approx_fast(out=rcp,
                                                     in_=accs_p[:, 5:6])
                    ot = outp.tile([P, N], BF16)
                    nc.vector.tensor_scalar_mul(out=ot, in0=r2_p,
                                                scalar1=rcp)
                    nc.sync.dma_start(out=out[qsl_p, :], in_=ot)

                with tc.tile_pool(name="sim_ps", bufs=2,
                                  space="PSUM") as simps:
                    for qb in range(NQB):
                        qsl = slice(qb * P, (qb + 1) * P)
                        r2 = r2p.tile([P, N], BF16)
                        accs = accp.tile([P, 12], F32)
                        pats = blk_pats[qb % len(blk_pats)]
                        for hh in range(2):
                            csl = slice(hh * 2048, (hh + 1) * 2048)
                            sim = simps.tile([P, 2048], F32)
                            for m in range(4):
                                lo = hh * 2048 + m * 512
                                nc.tensor.matmul(
                                    sim[:, m * 512:(m + 1) * 512],
                                    qT[:, qsl], kT[:, lo:lo + 512],
                                    start=True, stop=True)
                            # Per-half drain, 2 passes (the HW allows only a
                            # single PSUM operand per DVE instruction, and
                            # the Pool engine has neither PSUM access nor
                            # tensor_scalar support): relu PSUM->SBUF bf16
                            # on DVE or Act, then square + rowsum-accum in
                            # place on DVE (stt) or Act (Square w/
                            # accum_out).
                            reng, seng = pats[hh]
                            if reng == "d":
                                nc.vector.tensor_scalar_max(
                                    out=r2[:, csl], in0=sim, scalar1=0.0)
                            else:
                                nc.scalar.activation(r2[:, csl], sim,
                                                     AF.Relu)
                            if seng == "d":
                                nc.vector.scalar_tensor_tensor(
                                    out=r2[:, csl], in0=r2[:, csl],
                                    scalar=0.0, in1=r2[:, csl],
                                    op0=OP.add, op1=OP.mult,
                                    accum_out=accs[:, hh:hh + 1])
                            else:
                                nc.scalar.activation(
                                    r2[:, csl], r2[:, csl], AF.Square,
                                    accum_out=accs[:, hh:hh + 1])
                            if hh == 0 and pending is not None:
                                _finish(pending)
                                pending = None
                        pending = (qsl, r2, accs)
                    _finish(pending)
                    pending = None
            if _rep_cm is not None:
                _rep_cm.__exit__(None, None, None)
    nc.compile()
    return nc


def _prepare_in_maps(x, ln_w, ln_b, w_qk, b_qk, gamma, beta):
    x = np.asarray(x, np.float32)
    ln_w = np.asarray(ln_w, np.float32)
    ln_b = np.asarray(ln_b, np.float32)
    w_qk = np.asarray(w_qk, np.float32)
    b_qk = np.asarray(b_qk, np.float32)
    gamma = np.asarray(gamma, np.float32)
    beta = np.asarray(beta, np.float32)

    wp = (w_qk * ln_w[None, :]).astype(np.float64)
    bias_fold = (b_qk.astype(np.float64) + wp @ ln_b.astype(np.float64))
    svec = wp.sum(axis=1)  # (128,)
    scale = 1.0 / np.sqrt(np.float64(N))
    aff = np.stack([gamma[0] * scale, beta[0] * scale, gamma[1], beta[1]],
                   axis=1).astype(np.float32)  # (128, 4)

    wT = np.ascontiguousarray(wp.T).astype(BF16_NP)  # (512, 128)
    svec_bf = svec.astype(BF16_NP).reshape(1, P)
    bias_f = bias_fold.astype(np.float32).reshape(P, 1)

    in_maps = []
    for c in range(NCORES):
        b, h = c // 2, c % 2
        xt = x[b].T
        if h:
            xt = np.roll(xt, -HALF, axis=1)
        xt = np.ascontiguousarray(xt).astype(BF16_NP)
        in_maps.append({
            "xT": xt,
            "wT": wT,
            "svec": svec_bf,
            "biasf": bias_f,
            "aff": aff,
        })
    return in_maps


def _run(in_maps, trace=False):
    if "nc" not in _CACHE:
        _CACHE["nc"] = build_bass()
    nc = _CACHE["nc"]
    res = run_bass_kernel_spmd(nc, in_maps, core_ids=list(range(NCORES)),
                               trace=trace)
    return res


def kernel(x, ln_w, ln_b, w_qk, b_qk, gamma, beta, _trace=False):
    in_maps = _prepare_in_maps(x, ln_w, ln_b, w_qk, b_qk, gamma, beta)
    res = _run(in_maps, trace=_trace)
    out = np.empty((B, N, N), np.float32)
    for c in range(NCORES):
        b, h = c // 2, c % 2
        o = np.asarray(res.results[c]["out"]).astype(np.float32)
        if h:
            o = np.roll(o, HALF, axis=1)
        out[b, h * HALF:(h + 1) * HALF, :] = o
    if _trace:
        return out, res
    return out


# revision 33
# speedup vs baseline: 86.7869x; 1.2042x over previous
"""Self-contained Trainium2 Bass kernel for nn_Attn_3375844295368.

Reference computation (per batch b):
    normed = LayerNorm(x[b])                      # (4096, 512)
    qk = silu(normed @ W.T + bias)                # (4096, 128)
    q = (qk*g0 + b0) / sqrt(N); k = qk*g1 + b1
    sim = q @ k.T                                 # (4096, 4096)
    attn = relu(sim)^2 / (rowsum + 1e-6)

Sharding: 8 cores = 4 batches x 2 query-halves.  Each core receives the
full x[b] transposed (dim-major, bf16) and rolled so its query half is
always columns 0..2047 -> all cores run one identical SPMD graph,
outputs are un-rolled on the host.

Design (v2, engine-balanced, bf16 output):
  - Output is stored as bf16 (halves the dominant HBM store traffic);
    the host upcasts to f32.  DMA floor/core: 16.8MB out + 4.2MB in
    at ~360GB/s  ~= 63us/rep.
  - Phase 2 PSUM drain split across engines: DVE does relu^2+rowsum in
    a single scalar_tensor_tensor (max0, mult, accum) on quarter 0 of
    each 128-row query block; Act relu's quarters 1..3 (PSUM f32 ->
    SBUF bf16); Pool (no PSUM port!) squares+accumulates those in
    place.  Row normalization: rowsum+eps reduce on Pool, reciprocal
    on DVE, scale as a 4x-mode DVE tensor_scalar (bf16 all-SBUF).
  - Phase 1 (LN stats via all-ones matmul broadcast trick, mean folded
    into the linear as a rank-1 update, rstd applied post-linear) is
    spread: DVE negmu/var/qksc + kT/qT offsetscale (4x), Act rstd via
    Abs_reciprocal_sqrt + silu, Pool x^2 tiles and mu^2.
  - PSUM: stats 2 banks + z 2 banks + sim quarters [P,1024]x2 bufs
    4 banks = 8 banks total, all pools open for the whole kernel so
    consecutive reps (and phases) overlap.
"""

import sys

sys.path.insert(0, "/opt/trn_rl_repo")

import numpy as np
import ml_dtypes

import concourse.bass as bass
import concourse.bacc as bacc
import concourse.tile as tile
from concourse import mybir
from concourse.bass_utils import run_bass_kernel_spmd

B, N, DIM, QK = 4, 4096, 512, 128
NCORES = 8
HALF = N // 2
P = 128
NT = N // 512  # 8 column tiles of 512
NQB = HALF // P  # 16 query blocks per core
NQ = 4  # sim quarters per block
QW = N // NQ  # 1024 cols per quarter
LN_EPS = 1e-5
DEN_EPS = 1e-6
F32 = mybir.dt.float32
BF16 = mybir.dt.bfloat16
BF16_NP = ml_dtypes.bfloat16

_CACHE = {}


def build_bass(reps=1, dyn_reps=False, use_ars=True,
               xq_eng=("dve", "dve", "pool", "pool"),
               blk_pats=(("aa", "ad"), ("da", "ad")),
               r2_bufs=3, out_bufs=3, xload_chunks=4):
    nc = bacc.Bacc()
    xT = nc.declare_dram_parameter("xT", [DIM, N], BF16, isOutput=False)
    wT = nc.declare_dram_parameter("wT", [DIM, QK], BF16, isOutput=False)
    svec = nc.declare_dram_parameter("svec", [1, P], BF16, isOutput=False)
    biasf = nc.declare_dram_parameter("biasf", [P, 1], F32, isOutput=False)
    aff = nc.declare_dram_parameter("aff", [P, 4], F32, isOutput=False)
    if dyn_reps:
        nreps = nc.declare_dram_parameter("nreps", [1, 1], mybir.dt.int32,
                                          isOutput=False)
    out = nc.declare_dram_parameter("out", [HALF, N], BF16, isOutput=True)

    AF = mybir.ActivationFunctionType
    OP = mybir.AluOpType

    with tile.TileContext(nc) as tc:
        with tc.tile_pool(name="consts", bufs=1) as consts, \
             tc.tile_pool(name="xin", bufs=1) as xinp, \
             tc.tile_pool(name="xsq", bufs=6) as xsqp, \
             tc.tile_pool(name="qksc", bufs=1) as qkscp, \
             tc.tile_pool(name="st_sb", bufs=4) as stsb, \
             tc.tile_pool(name="accp", bufs=4) as accp, \
             tc.tile_pool(name="r2p", bufs=r2_bufs) as r2p, \
             tc.tile_pool(name="outp", bufs=out_bufs) as outp:
            wts = consts.tile([P, 4, QK], BF16)
            nc.sync.dma_start(out=wts, in_=wT.rearrange("(c p) m -> p c m", p=P))
            onest = consts.tile([P, P], BF16)
            nc.vector.memset(onest, 1.0 / DIM)
            svect = consts.tile([1, P], BF16)
            nc.sync.dma_start(out=svect, in_=svec[:])
            nones1 = consts.tile([1, P], BF16)
            nc.vector.memset(nones1, -1.0)
            biast = consts.tile([P, 1], F32)
            nc.sync.dma_start(out=biast, in_=biasf[:])
            afft = consts.tile([P, 4], F32)
            nc.sync.dma_start(out=afft, in_=aff[:])
            epst = consts.tile([P, 1], F32)
            nc.vector.memset(epst, LN_EPS)
            dent = consts.tile([P, 1], F32)
            nc.vector.memset(dent, DEN_EPS)
            zerot = consts.tile([P, 1], F32)
            nc.vector.memset(zerot, 0.0)
            kT = consts.tile([P, N], BF16)
            qT = consts.tile([P, HALF], BF16)

            _rep_cm = None
            if dyn_reps:
                nrt = consts.tile([1, 1], mybir.dt.int32)
                nc.sync.dma_start(out=nrt, in_=nreps[:])
                _regs = bass.RegisterHandles([
                    nc.engines[e].alloc_register(f"nreps_{e.name}")
                    for e in mybir.ALL_ENGINES])
                nc.regs_load(_regs, nrt[0:1, 0:1])
                rv = nc.snap(_regs, min_val=1, max_val=1024)
                _rep_cm = tc.For_i(0, rv, 1,
                                   hint_engines=(mybir.EngineType.PE,
                                                 mybir.EngineType.DVE,
                                                 mybir.EngineType.Activation,
                                                 mybir.EngineType.Pool))
                _rep_cm.__enter__()
            for _rep in range(reps):
                # x input: 4 dim-chunks x 2 column halves, interleaved so the
                # first phase-1 tiles can start after ~1/2 of the load.
                xins = []
                for c in range(4):
                    xi = xinp.tile([P, N], BF16, tag=f"xin{c}")
                    xins.append(xi)
                cw = N // xload_chunks
                for h in range(xload_chunks):
                    for c in range(4):
                        nc.sync.dma_start(
                            out=xins[c][:, h * cw:(h + 1) * cw],
                            in_=xT[c * P:(c + 1) * P, h * cw:(h + 1) * cw])

                # ---------- phase 1A: LN stats + linear (x rstd) ------------
                # Two stages so the Act engine switches function tables only
                # twice per rep (ARS set in A, Silu set in B; Relu for phase
                # 2 is in every set).  Separate 2-buf PSUM pools (6 banks)
                # close before phase 2 opens its 8-bank sim pool; with the
                # For_i all-engine barrier per rep there is no cross-rep
                # overlap to preserve.
                qkscs = []
                with tc.tile_pool(name="mu_ps", bufs=2, space="PSUM") as mups, \
                     tc.tile_pool(name="s2_ps", bufs=2, space="PSUM") as s2ps, \
                     tc.tile_pool(name="z_psp", bufs=2, space="PSUM") as zps:
                    for t in range(NT):
                        sl = slice(t * 512, (t + 1) * 512)
                        # mean & E[x^2]; the all-ones(1/512) stationary both
                        # reduces over dim and broadcasts to all partitions
                        mu_ps = mups.tile([P, 512], F32)
                        s2_ps = s2ps.tile([P, 512], F32)
                        z_ps = zps.tile([P, 512], F32)
                        for c in range(4):
                            nc.tensor.matmul(mu_ps, onest, xins[c][:, sl],
                                             start=(c == 0), stop=(c == 3))
                        for c in range(4):
                            xq = xsqp.tile([P, 512], BF16)
                            if xq_eng[c] == "pool":
                                nc.gpsimd.tensor_mul(xq, xins[c][:, sl],
                                                     xins[c][:, sl])
                            else:
                                nc.vector.tensor_mul(xq, xins[c][:, sl],
                                                     xins[c][:, sl])
                            nc.tensor.matmul(s2_ps, onest, xq,
                                             start=(c == 0), stop=False)
                        # z = W' @ x  (+ svec (x) -mu rank-1 LN-mean fold)
                        for c in range(4):
                            nc.tensor.matmul(z_ps, wts[:, c, :],
                                             xins[c][:, sl],
                                             start=(c == 0), stop=False)
                        negmu = stsb.tile([P, 512], BF16)
                        nc.vector.tensor_scalar_mul(negmu, mu_ps, -1.0)
                        # -mu^2 folded into s2 PSUM as a rank-1 update
                        # (negative all-ones stationary x mu^2 row), so
                        # rstd = ARS(s2_ps + eps) straight from PSUM
                        musqp = stsb.tile([P, 512], BF16)
                        nc.vector.tensor_mul(musqp, negmu, negmu)
                        nc.tensor.matmul(s2_ps, nones1, musqp[0:1, :],
                                         start=False, stop=True)
                        nc.tensor.matmul(z_ps, svect, negmu[0:1, :],
                                         start=False, stop=True)
                        rstd = stsb.tile([P, 512], F32)
                        if use_ars:
                            nc.scalar.activation(rstd, s2_ps,
                                                 AF.Abs_reciprocal_sqrt,
                                                 bias=epst)
                        else:
                            stdv = stsb.tile([P, 512], F32)
                            nc.scalar.activation(stdv, s2_ps, AF.Sqrt,
                                                 bias=epst)
                            nc.vector.reciprocal_approx_fast(out=rstd,
                                                             in_=stdv)
                        qksc = qkscp.tile([P, 512], F32, tag=f"qksc{t}")
                        nc.vector.scalar_tensor_tensor(
                            out=qksc, in0=z_ps, scalar=1.0, in1=rstd,
                            op0=OP.mult, op1=OP.mult)
                        qkscs.append(qksc)
                # ---------- phase 1B: silu + offsetscale --------------------
                # gate (==1.0) depends on the last stage-A qksc so the
                # scheduler cannot interleave Silu with ARS on the Act
                # engine (each interleave costs a 1283ns act-table reload)
                gate = stsb.tile([P, 1], F32)
                nc.vector.tensor_scalar(
                    out=gate, in0=qkscs[NT - 1][:, 0:1],
                    scalar1=0.0, scalar2=1.0, op0=OP.mult, op1=OP.add)
                for t in range(NT):
                    sl = slice(t * 512, (t + 1) * 512)
                    qka = stsb.tile([P, 512], F32)
                    nc.scalar.activation(qka, qkscs[t], AF.Silu,
                                         bias=biast, scale=gate)
                    nc.vector.tensor_scalar(
                        out=kT[:, sl], in0=qka,
                        scalar1=afft[:, 2:3], scalar2=afft[:, 3:4],
                        op0=OP.mult, op1=OP.add)
                    if t < NT // 2:
                        nc.vector.tensor_scalar(
                            out=qT[:, sl], in0=qka,
                            scalar1=afft[:, 0:1], scalar2=afft[:, 1:2],
                            op0=OP.mult, op1=OP.add)

                # ---------- phase 2: attention + relu^2 row-normalize -------
                # Normalize+scale+store of block qb-1 is emitted after block
                # qb's drains (software pipelining): DVE/Pool run in order,
                # so this keeps them from stalling on the cross-engine
                # rowsum -> reciprocal -> scale chain of the current block.
                pending = None

                def _finish(pend):
                    qsl_p, r2_p, accs_p = pend
                    # rowsum + eps in one tiny DVE stt (Pool's Q7 dispatch
                    # overhead would put it in the per-block critical path),
                    # then reciprocal + 4x-mode scale on DVE
                    nc.vector.scalar_tensor_tensor(
                        out=accs_p[:, 5:6], in0=accs_p[:, 0:1],
                        scalar=dent[:], in1=accs_p[:, 1:2],
                        op0=OP.add, op1=OP.add)
                    rcp = accp.tile([P, 1], F32)
                    nc.vector.reciprocal_approx_fast(out=rcp,
                                                     in_=accs_p[:, 5:6])
                    ot = outp.tile([P, N], BF16)
                    nc.vector.tensor_scalar_mul(out=ot, in0=r2_p,
                                                scalar1=rcp)
                    nc.sync.dma_start(out=out[qsl_p, :], in_=ot)

                with tc.tile_pool(name="sim_ps", bufs=2,
                                  space="PSUM") as simps:
                    for qb in range(NQB):
                        qsl = slice(qb * P, (qb + 1) * P)
                        r2 = r2p.tile([P, N], BF16)
                        accs = accp.tile([P, 12], F32)
                        pats = blk_pats[qb % len(blk_pats)]
                        for hh in range(2):
                            csl = slice(hh * 2048, (hh + 1) * 2048)
                            sim = simps.tile([P, 2048], F32)
                            for m in range(4):
                                lo = hh * 2048 + m * 512
                                nc.tensor.matmul(
                                    sim[:, m * 512:(m + 1) * 512],
                                    qT[:, qsl], kT[:, lo:lo + 512],
                                    start=True, stop=True)
                            # Per-half drain, 2 passes (the HW allows only a
                            # single PSUM operand per DVE instruction, and
                            # the Pool engine has neither PSUM access nor
                            # tensor_scalar support): relu PSUM->SBUF bf16
                            # on DVE or Act, then square + rowsum-accum in
                            # place on DVE (stt) or Act (Square w/
                            # accum_out).
                            reng, seng = pats[hh]
                            if reng == "d":
                                nc.vector.tensor_scalar_max(
                                    out=r2[:, csl], in0=sim, scalar1=0.0)
                            else:
                                nc.scalar.activation(r2[:, csl], sim,
                                                     AF.Relu)
                            if seng == "d":
                                nc.vector.scalar_tensor_tensor(
                                    out=r2[:, csl], in0=r2[:, csl],
                                    scalar=0.0, in1=r2[:, csl],
                                    op0=OP.add, op1=OP.mult,
                                    accum_out=accs[:, hh:hh + 1])
                            else:
                                nc.scalar.activation(
                                    r2[:, csl], r2[:, csl], AF.Square,
                                    accum_out=accs[:, hh:hh + 1])
                            if hh == 0 and pending is not None:
                                _finish(pending)
                                pending = None
                        pending = (qsl, r2, accs)
                    _finish(pending)
                    pending = None
            if _rep_cm is not None:
                _rep_cm.__exit__(None, None, None)
    nc.compile()
    return nc


def _prepare_in_maps(x, ln_w, ln_b, w_qk, b_qk, gamma, beta):
    x = np.asarray(x, np.float32)
    ln_w = np.asarray(ln_w, np.float32)
    ln_b = np.asarray(ln_b, np.float32)
    w_qk = np.asarray(w_qk, np.float32)
    b_qk = np.asarray(b_qk, np.float32)
    gamma = np.asarray(gamma, np.float32)
    beta = np.asarray(beta, np.float32)

    wp = (w_qk * ln_w[None, :]).astype(np.float64)
    bias_fold = (b_qk.astype(np.float64) + wp @ ln_b.astype(np.float64))
    svec = wp.sum(axis=1)  # (128,)
    scale = 1.0 / np.sqrt(np.float64(N))
    aff = np.stack([gamma[0] * scale, beta[0] * scale, gamma[1], beta[1]],
                   axis=1).astype(np.float32)  # (128, 4)

    wT = np.ascontiguousarray(wp.T).astype(BF16_NP)  # (512, 128)
    svec_bf = svec.astype(BF16_NP).reshape(1, P)
    bias_f = bias_fold.astype(np.float32).reshape(P, 1)

    in_maps = []
    for c in range(NCORES):
        b, h = c // 2, c % 2
        xt = x[b].T
        if h:
            xt = np.roll(xt, -HALF, axis=1)
        xt = np.ascontiguousarray(xt).astype(BF16_NP)
        in_maps.append({
            "xT": xt,
            "wT": wT,
            "svec": svec_bf,
            "biasf": bias_f,
            "aff": aff,
        })
    return in_maps


def _run(in_maps, trace=False):
    if "nc" not in _CACHE:
        _CACHE["nc"] = build_bass()
    nc = _CACHE["nc"]
    res = run_bass_kernel_spmd(nc, in_maps, core_ids=list(range(NCORES)),
                               trace=trace)
    return res


def kernel(x, ln_w, ln_b, w_qk, b_qk, gamma, beta, _trace=False):
    in_maps = _prepare_in_maps(x, ln_w, ln_b, w_qk, b_qk, gamma, beta)
    res = _run(in_maps, trace=_trace)
    out = np.empty((B, N, N), np.float32)
    for c in range(NCORES):
        b, h = c // 2, c % 2
        o = np.asarray(res.results[c]["out"]).astype(np.float32)
        if h:
            o = np.roll(o, HALF, axis=1)
        out[b, h * HALF:(h + 1) * HALF, :] = o
    if _trace:
        return out, res
    return out
